# revision 44
# baseline (speedup 1.0000x reference)
"""Trainium2 Bass kernel for nn_Attention_43190191129190.

Model (per batch element b of 8):
    y   = x + dwconv3x3(x) + conv_b          (depthwise residual positional conv)
    qkv = y @ qkv_w.T ; split into q, k, v   (8 heads, dim 32)
    out = softmax(q k^T / sqrt(32)) v
    out = out @ out_w.T + out_b

Sharding: pure data-parallel, one batch element per NeuronCore (8 cores).

Per-core design (everything in transposed [C, N] space so the depthwise conv
is 9 diagonal matmuls and q^T/k^T come out in the layout the S^T matmul wants):

  1. x [1024,256] -> PE transpose -> x^T zero-padded to [C, 34, 34] in SBUF.
  2. conv: per 128-channel tile, 9 matmuls with diagonal weight matrices
     (stationary = diag(conv_w tap), moving = shifted window of padded x^T),
     accumulated in PSUM; +1.0 folded into center tap (residual); bias via a
     K=1 matmul with a ones row.  -> y^T [c, n] in SBUF.
  3. q^T,k^T [feature, token]: stationary = qkv_w^T chunks, moving = y^T.
     Head h lives at partition offset 32*(h%4) of feature tile h//4.
  4. v [token, feature] with a per-head ones column interleaved ([v_h|1]):
     stationary = y^T chunks, moving = qkv_w^T.
  5. Per head pair (two heads with different h%4 so their S^T matmuls pack
     into different 32-row groups of the PE array):
       S^T[m,n] = k_h^T.T @ q_h^T via K=32 row-tiled matmuls;
       exp on ScalarE straight from PSUM (scale=1/sqrt(32) folded in, no max
       subtraction -- S is in [-11, 11] for this input distribution);
       PV: stationary = [v_h|1] (M=33), moving = exp(S^T) tiles, accumulated
       over the 8 m-chunks into psum rows 0:33 (fp32r requires a partition-0
       dst); the ones column yields the softmax denominators in row 32.
       The PV matmuls lag the exp stream so a blocked PV (pair boundary)
       never stalls ScalarE, and each pair's psum is evacuated by a single
       DVE copy so the slot frees immediately.
       Normalization: reciprocal(sums), broadcast to 32 partitions (DMA
       round-trip through a DRAM scratch row -- SBUF APs cannot have step-0
       partitions and gpsimd partition_broadcast misreads on HW), one vector
       multiply; heads whose attn^T rows are not 0:32 are repositioned with
       a SBUF->SBUF DMA (which, unlike DVE, can shift partitions).  The last
       pair instead broadcasts on the now-idle PE and evacuates via ScalarE
       to shorten the tail.
  6. out-projection: stationary = attn^T chunks, moving = out_w^T; the
     chunk-0 half runs mid-kernel into an SBUF staging tile, chunk-1 + bias
     (K=1 ones-row matmul) + the staged half finish the tail.

All matmuls use float32r (full-rate fp32 PE mode); accumulation is fp32 PSUM.
Remaining work (v projection, q/k feature tiles 1 and 3, chunk-0 projection)
is interleaved one slice per m-step into the pair loops so the in-order PE
queue fills DMA-gated gaps instead of delaying the first exp.
"""

import os

import numpy as np

import concourse.bass as bass
import concourse.tile as tile
from concourse import bacc, mybir
from concourse.bass_utils import run_bass_kernel_spmd

F32 = mybir.dt.float32
F32R = mybir.dt.float32r
AF = mybir.ActivationFunctionType

B, N, C = 8, 1024, 256
HEADS, DH = 8, 32
SCALE = DH ** -0.5
PAD = 34  # 32x32 spatial grid with 1-px halo

TAPS = [(ky, kx) for ky in range(3) for kx in range(3)]
# order: first two pairs complete attn^T chunk 0 (heads 0-3); last pair has a
# row-0 head (4) so only one tail DMA-repositioning remains
PAIRS = [(1, 3), (0, 2), (5, 7), (4, 6)]


def build_nc(debug_dump=False):
    nc = bacc.Bacc("TRN2", target_bir_lowering=False, debug=False, num_devices=8)

    x_d = nc.dram_tensor("x", (N, C), F32, kind="ExternalInput").ap()
    qkvwT_d = nc.dram_tensor("qkv_wT", (C, 3 * C), F32R, kind="ExternalInput").ap()
    outwT_d = nc.dram_tensor("out_wT", (C, C), F32R, kind="ExternalInput").ap()
    diag_d = nc.dram_tensor("conv_diag", (2, 9, 128, 128), F32R, kind="ExternalInput").ap()
    convb_d = nc.dram_tensor("conv_b_r", (1, C), F32R, kind="ExternalInput").ap()
    outb_d = nc.dram_tensor("out_b_r", (1, C), F32R, kind="ExternalInput").ap()
    ones_d = nc.dram_tensor("ones_row", (1, N), F32R, kind="ExternalInput").ap()
    id_d = nc.dram_tensor("id128", (128, 128), F32, kind="ExternalInput").ap()
    out_d = nc.dram_tensor("out", (N, C), F32, kind="ExternalOutput").ap()
    dbg = {}
    if debug_dump:
        for name, shape in (
            ("d_yT", (128, 2, N)), ("d_qT", (128, 2, N)), ("d_kT", (128, 2, N)),
            ("d_v", (128, 8, 8 * 33)), ("d_attnT", (128, 2, N)),
        ):
            dbg[name] = nc.dram_tensor(name, shape, F32, kind="ExternalOutput").ap()

    with tile.TileContext(nc) as tc:
        with (
            tc.tile_pool(name="const", bufs=1) as const,
            tc.tile_pool(name="xin", bufs=1) as xin_p,
            tc.tile_pool(name="big", bufs=1) as big,
            tc.tile_pool(name="pT", bufs=8) as ppool,
            tc.tile_pool(name="rs", bufs=2) as rs_p,
            tc.tile_pool(name="bc", bufs=2) as bc_p,
            tc.tile_pool(name="tmp", bufs=2) as tmp_p,
            tc.tile_pool(name="outs", bufs=3) as outs_p,
            tc.tile_pool(name="dscr", bufs=4, space="DRAM") as dram_p,
            tc.tile_pool(name="pst", bufs=2, space="PSUM") as pst,
            tc.tile_pool(name="ppv", bufs=1, space="PSUM") as ppv,
        ):
            # ---- DMAs: id128 + x tiles first (startup critical path),
            # weights after; x loads spread over three DGE queues
            id_sb = const.tile([128, 128], F32, tag="id")
            nc.sync.dma_start(id_sb, id_d)
            xins = []
            _dma_engines = [nc.sync, nc.scalar, nc.sync, nc.gpsimd]
            for nt in range(8):
                xin = xin_p.tile([128, C], F32, tag=f"xin{nt}", name=f"xin{nt}")
                _dma_engines[nt % 4].dma_start(xin, x_d[nt * 128:(nt + 1) * 128, :])
                xins.append(xin)
            diag_sb = const.tile([128, 18, 128], F32R, tag="diag")
            nc.sync.dma_start(diag_sb, diag_d.rearrange("ct t p f -> p (ct t) f"))
            convb_sb = const.tile([1, C], F32R, tag="convb")
            nc.sync.dma_start(convb_sb, convb_d)
            ones_sb = const.tile([1, N], F32R, tag="ones")
            nc.sync.dma_start(ones_sb, ones_d)
            qkvwT_sb = const.tile([128, 2, 3 * C], F32R, tag="qkvwT")
            nc.sync.dma_start(qkvwT_sb, qkvwT_d.rearrange("(kc p) f -> p kc f", p=128))
            outwT_sb = const.tile([128, 2, C], F32R, tag="outwT")
            nc.sync.dma_start(outwT_sb, outwT_d.rearrange("(kc p) f -> p kc f", p=128))
            outb_sb = const.tile([1, C], F32R, tag="outb")
            nc.sync.dma_start(outb_sb, outb_d)
            zerob_sb = const.tile([128, 1], F32, tag="zerob")
            nc.vector.memset(zerob_sb, 0.0)
            # dummy exp: hoists the ~2.7us exp_and_others ACT table load into
            # the idle startup window (it would otherwise fire at the first
            # real exp, delaying the critical ScalarE stream; the set also
            # contains Copy, so the alternated ScalarE copies share it)
            warm_sb = const.tile([1, 1], F32, tag="warm")
            nc.scalar.activation(
                warm_sb, zerob_sb[0:1, 0:1], AF.Exp,
                bias=zerob_sb[0:1], scale=1.0,
            )
            # all-ones strip on every partition (PE broadcast stationary must
            # share its base partition with the moving operand)
            onesp_sb = const.tile([128, 32], F32R, tag="onesp")
            nc.gpsimd.memset(onesp_sb.bitcast(mybir.dt.uint32), 0x3F800000)

            # ---- persistent activations ----
            xpadT = big.tile([128, 2, PAD * PAD], F32R, tag="xpadT")
            # zero only the 1-px halo ring (interior is fully overwritten);
            # via a uint32 view: walrus rejects Memset with f32r dtype
            xpv = xpadT.bitcast(mybir.dt.uint32).rearrange(
                "p ct (h w) -> p ct h w", h=PAD
            )
            nc.gpsimd.memset(xpv[:, :, 0, :], 0)
            nc.gpsimd.memset(xpv[:, :, PAD - 1, :], 0)
            nc.gpsimd.memset(xpv[:, :, :, 0], 0)
            nc.gpsimd.memset(xpv[:, :, :, PAD - 1], 0)
            yT = big.tile([128, 2, N], F32R, tag="yT")
            qT = big.tile([128, 2, N], F32R, tag="qT")
            kT = big.tile([128, 2, N], F32R, tag="kT")
            vsb = big.tile([128, 8, 8 * 33], F32R, tag="v")
            # 1.0 everywhere (ones columns); v cols overwritten below
            nc.gpsimd.memset(vsb.bitcast(mybir.dt.uint32), 0x3F800000)
            attnT = big.tile([128, 2, N], F32R, tag="attnT")
            partial0 = big.tile([128, 8, C], F32, tag="partial0")

            # pre-attention psum evacuations alternate between DVE and
            # the (still idle) ScalarE so neither queue gates slot turnover
            _cp = [0]

            def copy_alt(dst, src_ap):
                _cp[0] += 1
                if _cp[0] % 2:
                    nc.vector.tensor_copy(dst, src_ap)
                else:
                    nc.scalar.copy(dst, src_ap)

            # ---- transpose x into padded x^T, conv interleaved ----
            def emit_transpose(nt):
                tp = pst.tile([128, 1024], F32, tag="ps", name="tp")
                for ct in range(2):
                    nc.tensor.transpose(
                        tp[:, 512 * ct: 512 * ct + 128],
                        xins[nt][:, 128 * ct: 128 * (ct + 1)],
                        id_sb,
                    )
                    dst = xpadT[:, ct, :].rearrange("p (h w) -> p h w", h=PAD)[
                        :, 1 + 4 * nt: 5 + 4 * nt, 1:33
                    ]
                    copy_alt(
                        dst,
                        tp[:, 512 * ct: 512 * ct + 128].rearrange(
                            "p (a b) -> p a b", a=4
                        ),
                    )

            # conv accumulators live in the (otherwise still idle) PV psum
            # slot so the transposes keep both pst slots
            cacc = ppv.tile([128, 2048], F32, tag="pv", name="cacc")

            def emit_conv_half(ct, j):
                cps = cacc[:, ct * 1024:(ct + 1) * 1024]
                view = xpadT[:, ct, :].rearrange("p (h w) -> p h w", h=PAD)
                for t, (ky, kx) in enumerate(TAPS):
                    nc.tensor.matmul(
                        cps[:, j * 512:(j + 1) * 512],
                        lhsT=diag_sb[:, ct * 9 + t, :],
                        rhs=view[:, ky + 16 * j: ky + 16 * j + 16, kx: kx + 32],
                        start=(t == 0),
                        stop=False,
                    )
                nc.tensor.matmul(
                    cps[:, j * 512:(j + 1) * 512],
                    lhsT=convb_sb[0:1, 128 * ct: 128 * (ct + 1)],
                    rhs=ones_sb[0:1, j * 512:(j + 1) * 512],
                    start=False,
                    stop=True,
                )

            # conv j=0 only needs padded rows 0..18 (x tiles 0..4), so its
            # matmuls fill the PE gaps while tiles 5..7 still stream in
            for nt in range(5):
                emit_transpose(nt)
            emit_conv_half(0, 0)
            emit_conv_half(1, 0)
            for nt in range(5, 8):
                emit_transpose(nt)
            for ct in range(2):
                emit_conv_half(ct, 1)
                copy_alt(yT[:, ct, :], cacc[:, ct * 1024:(ct + 1) * 1024])

            # ---- q^T / k^T feature tiles (heads 0-3 now; 4-7 interleaved
            # into the first pair's m-loop) ----
            def emit_qk(ft):
                dstT, dc = (qT, ft) if ft < 2 else (kT, ft - 2)
                fofs = 0 if ft < 2 else 256
                qps = pst.tile([128, 1024], F32, tag="ps", name="qps")
                for j in range(2):
                    for kc in range(2):
                        nc.tensor.matmul(
                            qps[:, j * 512:(j + 1) * 512],
                            lhsT=qkvwT_sb[:, kc, fofs + dc * 128: fofs + (dc + 1) * 128],
                            rhs=yT[:, kc, j * 512:(j + 1) * 512],
                            start=(kc == 0),
                            stop=(kc == 1),
                        )
                nc.vector.tensor_copy(dstT[:, dc, :], qps)

            def emit_v(nt):
                vps = pst.tile([128, 1024], F32, tag="ps", name="vps")
                for kc in range(2):
                    nc.tensor.matmul(
                        vps[:, 0:256],
                        lhsT=yT[:, kc, nt * 128:(nt + 1) * 128],
                        rhs=qkvwT_sb[:, kc, 512:768],
                        start=(kc == 0),
                        stop=(kc == 1),
                    )
                vv = vsb[:, nt, :].rearrange("p (hh c) -> p hh c", c=33)
                sv = vps[:, 0:256].rearrange("p (hh c) -> p hh c", c=32)
                copy_alt(vv[:, :, 0:32], sv)  # [v_h | 1] per head

            def emit_proj0(nt):
                opsA = pst.tile([128, 1024], F32, tag="ps", name="opsA")
                nc.tensor.matmul(
                    opsA[:, 0:256],
                    lhsT=attnT[:, 0, nt * 128:(nt + 1) * 128],
                    rhs=outwT_sb[:, 0, :],
                    start=True,
                    stop=True,
                )
                nc.vector.tensor_copy(partial0[:, nt, :], opsA[:, 0:256])

            emit_qk(0)
            emit_qk(2)
            emit_qk(1)
            emit_qk(3)
            for nt in range(8):
                emit_v(nt)

            # chunk-0 out-projection interleaved one tile per m-step into
            # the last pair's loop (chunk 0 is long since finished by then)
            def pair_extra(ip, m):
                if ip == 3:
                    emit_proj0(m)

            # ---- attention, head pair at a time ----
            for ip, (hA, hB) in enumerate(PAIRS):
                last_pair = ip == len(PAIRS) - 1
                pv = ppv.tile([128, 2048], F32, tag="pv")

                def emit_pv(m, pA, pB, pv=pv, hA=hA, hB=hB):
                    # PV: [v_h|1] stationary (M=33), exp(S^T) moving; fp32r
                    # dst must start at partition 0, so both heads land in
                    # rows 0:33 -- head A in psum banks 0-1, head B in 2-3.
                    for j in range(2):
                        for h, pT, cofs in ((hA, pA, 0), (hB, pB, 1024)):
                            nc.tensor.matmul(
                                pv[0:33, cofs + j * 512: cofs + j * 512 + 512],
                                lhsT=vsb[:, m, 33 * h: 33 * h + 33],
                                rhs=pT[:, j * 512:(j + 1) * 512],
                                start=(m == 0),
                                stop=(m == 7),
                            )

                lag = 1 if last_pair else 2
                pend = []  # (m, pA, pB) awaiting their PV matmuls
                for m in range(8):
                    stA = pst.tile([128, 1024], F32, tag="ps")
                    stB = pst.tile([128, 1024], F32, tag="ps")
                    # S^T matmuls: 2 heads packed in different 32-row groups
                    for j in range(2):
                        for h, st in ((hA, stA), (hB, stB)):
                            a = 32 * (h % 4)
                            hc = h // 4
                            nc.tensor.matmul(
                                st[:, j * 512:(j + 1) * 512],
                                lhsT=kT[a:a + 32, hc, m * 128:(m + 1) * 128],
                                rhs=qT[a:a + 32, hc, j * 512:(j + 1) * 512],
                                start=True,
                                stop=True,
                                tile_position=(a, 0),
                            )
                    pA = ppool.tile([128, 1024], F32R, tag="pT")
                    pB = ppool.tile([128, 1024], F32R, tag="pT")
                    nc.scalar.activation(pA, stA, AF.Exp, bias=zerob_sb, scale=SCALE)
                    nc.scalar.activation(pB, stB, AF.Exp, bias=zerob_sb, scale=SCALE)
                    pair_extra(ip, m)
                    pend.append((m, pA, pB))
                    if len(pend) > lag:
                        emit_pv(*pend.pop(0))
                for e in pend:
                    emit_pv(*e)

                # ---- softmax normalization ----
                rs = rs_p.tile([128, 2048], F32, tag="rs")
                bc = bc_p.tile([128, 2048], F32, tag="bc")
                if not last_pair:
                    # evacuate pv with one DVE copy (frees the psum slot for
                    # the next pair), then normalize off-slot
                    pc = tmp_p.tile([128, 2048], F32, tag="pc", name="pc")
                    nc.vector.tensor_copy(pc[0:33, :], pv[0:33, :])
                    for h, cofs in ((hA, 0), (hB, 1024)):
                        nc.vector.reciprocal(
                            rs[32:33, cofs:cofs + 1024], pc[32:33, cofs:cofs + 1024]
                        )
                        # broadcast the reciprocal row to 32 partitions via a
                        # DRAM scratch row (SBUF step-0 partition APs are
                        # illegal; partition_broadcast misreads on HW)
                        rsd = dram_p.tile([1, 1024], F32, tag="rsd", name="rsd")
                        nc.sync.dma_start(rsd, rs[32:33, cofs:cofs + 1024])
                        row = 32 * (h % 4)
                        ic = h // 4
                        nc.gpsimd.dma_start(
                            out=bc[row:row + 32, cofs:cofs + 1024],
                            in_=bass.AP(
                                tensor=rsd.tensor,
                                offset=rsd.offset,
                                ap=[[0, 32]] + list(rsd.ap[1:]),
                            ),
                        )
                        if row == 0:
                            nc.vector.tensor_mul(
                                attnT[0:32, ic, :],
                                pc[0:32, cofs:cofs + 1024],
                                bc[0:32, cofs:cofs + 1024],
                            )
                        else:
                            # reposition to the head's attn^T rows (DMA can
                            # shift partitions; DVE cannot)
                            pcs = tmp_p.tile([128, 1024], F32, tag="pcs", name="pcs")
                            nc.sync.dma_start(
                                pcs[row:row + 32, :], pc[0:32, cofs:cofs + 1024]
                            )
                            nc.vector.tensor_mul(
                                attnT[row:row + 32, ic, :],
                                pcs[row:row + 32, :],
                                bc[row:row + 32, cofs:cofs + 1024],
                            )
                else:
                    # tail-optimized: broadcast on the now-idle PE (ones32
                    # stationary x reciprocal row), evacuate via ScalarE, and
                    # multiply straight from the pv psum (single psum operand)
                    rs2 = rs_p.tile([128, 2048], F32R, tag="rs2", name="rs2")
                    for h, cofs in ((hA, 0), (hB, 1024)):
                        nc.vector.reciprocal(
                            rs[32:33, cofs:cofs + 1024], pv[32:33, cofs:cofs + 1024]
                        )
                        # fp32r-round the reciprocal row on ScalarE (walrus
                        # requires fp32r-typed producers for matmul operands)
                        nc.scalar.copy(
                            rs2[32:33, cofs:cofs + 1024], rs[32:33, cofs:cofs + 1024]
                        )
                        bcp = pst.tile([128, 1024], F32, tag="ps", name="bcp")
                        for j in range(2):
                            nc.tensor.matmul(
                                bcp[0:32, j * 512:(j + 1) * 512],
                                lhsT=onesp_sb[32:33, :],
                                rhs=rs2[32:33, cofs + j * 512: cofs + j * 512 + 512],
                                start=True,
                                stop=True,
                            )
                        nc.scalar.copy(bc[0:32, cofs:cofs + 1024], bcp[0:32, :])
                        row = 32 * (h % 4)
                        ic = h // 4
                        if row == 0:
                            nc.vector.tensor_mul(
                                attnT[0:32, ic, :],
                                pv[0:32, cofs:cofs + 1024],
                                bc[0:32, cofs:cofs + 1024],
                            )
                        else:
                            pcs = tmp_p.tile([128, 1024], F32R, tag="pcs2", name="pcs")
                            nc.vector.tensor_mul(
                                pcs[0:32, :],
                                pv[0:32, cofs:cofs + 1024],
                                bc[0:32, cofs:cofs + 1024],
                            )
                            nc.sync.dma_start(
                                attnT[row:row + 32, ic, :], pcs[0:32, :]
                            )

            if debug_dump:
                nc.sync.dma_start(dbg["d_yT"], yT.bitcast(F32))
                nc.sync.dma_start(dbg["d_qT"], qT.bitcast(F32))
                nc.sync.dma_start(dbg["d_kT"], kT.bitcast(F32))
                nc.sync.dma_start(dbg["d_v"], vsb.bitcast(F32))
                nc.sync.dma_start(dbg["d_attnT"], attnT.bitcast(F32))

            # ---- out projection: chunk-1 half + bias + staged chunk-0 ----
            for nt in range(8):
                ops = pst.tile([128, 1024], F32, tag="ps")
                nc.tensor.matmul(
                    ops[:, 0:256],
                    lhsT=attnT[:, 1, nt * 128:(nt + 1) * 128],
                    rhs=outwT_sb[:, 1, :],
                    start=True,
                    stop=False,
                )
                nc.tensor.matmul(
                    ops[:, 0:256],
                    lhsT=ones_sb[0:1, 0:128],
                    rhs=outb_sb,
                    start=False,
                    stop=True,
                )
                osb = outs_p.tile([128, C], F32, tag="o")
                nc.vector.tensor_add(osb, ops[:, 0:256], partial0[:, nt, :])
                nc.sync.dma_start(out_d[nt * 128:(nt + 1) * 128, :], osb)

    nc.compile()
    return nc


_NC = None
LAST_RESULTS = None


def _host_prep(conv_w, conv_b, qkv_w, out_w, out_b):
    conv_w = np.asarray(conv_w, np.float32).reshape(C, 3, 3)
    diag = np.zeros((2, 9, 128, 128), np.float32)
    idx = np.arange(128)
    for ct in range(2):
        for t, (ky, kx) in enumerate(TAPS):
            d = conv_w[128 * ct: 128 * (ct + 1), ky, kx].copy()
            if (ky, kx) == (1, 1):
                d += 1.0  # residual connection folded into the center tap
            diag[ct, t, idx, idx] = d
    return {
        "qkv_wT": np.ascontiguousarray(np.asarray(qkv_w, np.float32).T),
        "out_wT": np.ascontiguousarray(np.asarray(out_w, np.float32).T),
        "conv_diag": diag,
        "conv_b_r": np.asarray(conv_b, np.float32).reshape(1, C),
        "out_b_r": np.asarray(out_b, np.float32).reshape(1, C),
        "ones_row": np.ones((1, N), np.float32),
        "id128": np.eye(128, dtype=np.float32),
    }


def kernel(x, conv_w, conv_b, qkv_w, out_w, out_b):
    global _NC, LAST_RESULTS
    if _NC is None:
        _NC = build_nc()
    x = np.asarray(x, np.float32)
    shared = _host_prep(conv_w, conv_b, qkv_w, out_w, out_b)
    in_maps = [{**shared, "x": np.ascontiguousarray(x[b])} for b in range(B)]
    trace = bool(int(os.environ.get("KERNEL_TRACE", "0")))
    try:
        res = run_bass_kernel_spmd(_NC, in_maps, core_ids=list(range(B)), trace=trace)
    except Exception:
        if not trace:
            raise
        # NTFF profiling unavailable (e.g. no antenv hook) -- run untraced
        res = run_bass_kernel_spmd(_NC, in_maps, core_ids=list(range(B)), trace=False)
    LAST_RESULTS = res
    return np.stack([res.results[b]["out"] for b in range(B)], axis=0)



# revision 45
# speedup vs baseline: 1.1948x; 1.1948x over previous
"""Trainium2 Bass kernel for nn_Attention_43190191129190.

Model (per batch element b of 8):
    y   = x + dwconv3x3(x) + conv_b          (depthwise residual positional conv)
    qkv = y @ qkv_w.T ; split into q, k, v   (8 heads, dim 32)
    out = softmax(q k^T / sqrt(32)) v
    out = out @ out_w.T + out_b

Sharding: pure data-parallel, one batch element per NeuronCore (8 cores).

Per-core design (transposed [C, N] space; see kernel_baseline.py.bak for the
ancestor).  v2 changes vs the baseline:

  * Attention is split over the QUERY dim into two 512-token halves.  All 4
    head pairs run on half 0, whose normalization + out-projection + store
    overlap half 1's attention; only half 1's last pair remains in the tail.
  * PSUM re-plan: pst 2x[128,1024] (S^T double-buffer), ppv 1x[128,1024]
    (PV accumulator, freed by a single DVE evacuation), paux 1x[128,1024]
    (conv ct0 / v / qk tiles 1,3 / softmax-broadcast / projection).
  * Softmax normalization with no DRAM round trip: evacuate pv -> pc (SBUF),
    PE-broadcast the sums row (ones stationary), DVE reciprocal from PSUM,
    DVE multiply; deferred past the next pair's first S^T so PE never waits.
  * ScalarE runs exps only (one [128,1024] exp per pair/m covering both
    heads); all other evacuations are on DVE (preamble ones may use ScalarE
    while it is still idle).
  * Input DMAs spread over the four HWDGE queues + the Pool SWDGE path.
"""

import os

import numpy as np

import concourse.bass as bass
import concourse.tile as tile
from concourse import bacc, mybir
from concourse.bass_utils import run_bass_kernel_spmd

F32 = mybir.dt.float32
F32R = mybir.dt.float32r
AF = mybir.ActivationFunctionType

B, N, C = 8, 1024, 256
HEADS, DH = 8, 32
SCALE = DH ** -0.5
PAD = 34  # 32x32 spatial grid with 1-px halo
HALF = 512

TAPS = [(ky, kx) for ky in range(3) for kx in range(3)]
# Head pairs per query half.  Half 0 must order so q/k feature tiles 1,3
# (emitted during its pair 1) precede any head >= 4.  Half 1 re-pairs so the
# final pair (0,4) writes attn^T rows 0:32 of both chunks directly -- no
# repositioning DMA in the kernel tail.
PAIRS_H = [
    [(1, 3), (0, 2), (5, 7), (4, 6)],
    [(1, 3), (5, 7), (2, 6), (0, 4)],
]


def build_nc(debug_dump=False):
    nc = bacc.Bacc("TRN2", target_bir_lowering=False, debug=False, num_devices=8)

    x_d = nc.dram_tensor("x", (N, C), F32R, kind="ExternalInput").ap()
    qkvwT_d = nc.dram_tensor("qkv_wT", (C, 3 * C), F32R, kind="ExternalInput").ap()
    outwT_d = nc.dram_tensor("out_wT", (C, C), F32R, kind="ExternalInput").ap()
    diag_d = nc.dram_tensor("conv_diag", (2, 9, 128, 128), F32R, kind="ExternalInput").ap()
    convb_d = nc.dram_tensor("conv_b_r", (1, C), F32R, kind="ExternalInput").ap()
    outb_d = nc.dram_tensor("out_b_r", (1, C), F32R, kind="ExternalInput").ap()
    id_d = nc.dram_tensor("id128", (128, 128), F32R, kind="ExternalInput").ap()
    out_d = nc.dram_tensor("out", (N, C), F32, kind="ExternalOutput").ap()
    dbg = {}
    if debug_dump:
        for name, shape in (
            ("d_yT", (128, 2, N)), ("d_qT", (128, 2, N)), ("d_kT", (128, 2, N)),
            ("d_v", (128, 8, 8 * 33)), ("d_attnT", (128, 2, N)),
        ):
            dbg[name] = nc.dram_tensor(name, shape, F32, kind="ExternalOutput").ap()

    with tile.TileContext(nc) as tc:
        with (
            tc.tile_pool(name="const", bufs=1) as const,
            tc.tile_pool(name="xin", bufs=1) as xin_p,
            tc.tile_pool(name="big", bufs=1) as big,
            tc.tile_pool(name="pT", bufs=8) as ppool,
            tc.tile_pool(name="rs", bufs=2) as rs_p,
            tc.tile_pool(name="tmp", bufs=2) as tmp_p,
            tc.tile_pool(name="tmp2", bufs=2) as tmp2_p,
            tc.tile_pool(name="outs", bufs=3) as outs_p,
            tc.tile_pool(name="pst", bufs=2, space="PSUM") as pst,
            tc.tile_pool(name="ppv", bufs=1, space="PSUM") as ppv,
            tc.tile_pool(name="paux", bufs=1, space="PSUM") as paux,
        ):
            # ---- input DMAs.  Critical path: id128 + x0..x4 + diag(ct0)
            # (transposes then conv j=0).  HWDGE is a single shared device
            # (~630ns serialization per DMA), so id128 and the noncritical
            # loads go via the parallel Pool SWDGE path; x0..x4 + the conv
            # diag lead the HWDGE order, weights follow.
            id_sb = const.tile([128, 128], F32R, tag="id")
            nc.gpsimd.dma_start(id_sb, id_d)
            xins = [None] * 8
            _xq = {0: nc.sync, 1: nc.scalar, 2: nc.sync, 3: nc.scalar,
                   4: nc.scalar, 5: nc.gpsimd, 6: nc.gpsimd, 7: nc.gpsimd}
            diag_sb = const.tile([128, 18, 128], F32R, tag="diag")
            diag_r = diag_d.rearrange("ct t p f -> p (ct t) f")

            def load_x(nt):
                xin = xin_p.tile([128, C], F32R, tag=f"xin{nt}", name=f"xin{nt}")
                _xq[nt].dma_start(xin, x_d[nt * 128:(nt + 1) * 128, :])
                xins[nt] = xin

            # HWDGE DGEs serialize in emission order (~630ns apiece) and the
            # DMA-engine pool drains transfers roughly in DGE-completion
            # order, so the emission order below IS the arrival order.  The
            # conv diag is split in four so each chunk lands just before its
            # taps run; q/k weights (first 512 features) precede v weights.
            qkvwT_sb = const.tile([128, 2, 3 * C], F32R, tag="qkvwT")
            qkvw_r = qkvwT_d.rearrange("(kc p) f -> p kc f", p=128)
            for nt in (0, 1, 2, 3):
                load_x(nt)
            nc.sync.dma_start(diag_sb[:, 0:5, :], diag_r[:, 0:5, :])
            load_x(4)
            nc.sync.dma_start(diag_sb[:, 5:9, :], diag_r[:, 5:9, :])
            nc.scalar.dma_start(qkvwT_sb[:, :, 0:512], qkvw_r[:, :, 0:512])
            nc.sync.dma_start(diag_sb[:, 9:14, :], diag_r[:, 9:14, :])
            nc.scalar.dma_start(diag_sb[:, 14:18, :], diag_r[:, 14:18, :])
            nc.sync.dma_start(qkvwT_sb[:, :, 512:768], qkvw_r[:, :, 512:768])
            for nt in (5, 6, 7):
                load_x(nt)
            convb_sb = const.tile([1, C], F32R, tag="convb")
            nc.gpsimd.dma_start(convb_sb, convb_d)
            ones_sb = const.tile([1, N], F32R, tag="ones")
            nc.gpsimd.memset(ones_sb.bitcast(mybir.dt.uint32), 0x3F800000)
            outwT_sb = const.tile([128, 2, C], F32R, tag="outwT")
            nc.gpsimd.dma_start(outwT_sb, outwT_d.rearrange("(kc p) f -> p kc f", p=128))
            outb_sb = const.tile([1, C], F32R, tag="outb")
            nc.gpsimd.dma_start(outb_sb, outb_d)
            zerob_sb = const.tile([128, 1], F32, tag="zerob")
            nc.vector.memset(zerob_sb, 0.0)
            # dummy exp: hoists the ~1.3us exp_and_others ACT table load into
            # the idle startup window (the set also contains Copy, so the
            # preamble ScalarE copies share it)
            warm_sb = const.tile([1, 1], F32, tag="warm")
            nc.scalar.activation(
                warm_sb, zerob_sb[0:1, 0:1], AF.Exp,
                bias=zerob_sb[0:1], scale=1.0,
            )
            # all-ones strip on every partition (PE broadcast stationary must
            # share its base partition with the moving operand)
            onesp_sb = const.tile([128, 32], F32R, tag="onesp")
            nc.gpsimd.memset(onesp_sb.bitcast(mybir.dt.uint32), 0x3F800000)

            # ---- persistent activations ----
            xpadT = big.tile([128, 2, PAD * PAD], F32R, tag="xpadT")
            # zero only the 1-px halo ring (interior is fully overwritten)
            xpv = xpadT.bitcast(mybir.dt.uint32).rearrange(
                "p ct (h w) -> p ct h w", h=PAD
            )
            nc.gpsimd.memset(xpv[:, :, 0, :], 0)
            nc.gpsimd.memset(xpv[:, :, PAD - 1, :], 0)
            nc.gpsimd.memset(xpv[:, :, :, 0], 0)
            nc.gpsimd.memset(xpv[:, :, :, PAD - 1], 0)
            yT = big.tile([128, 2, N], F32R, tag="yT")
            qT = big.tile([128, 2, N], F32R, tag="qT")
            kT = big.tile([128, 2, N], F32R, tag="kT")
            vsb = big.tile([128, 8, 8 * 33], F32R, tag="v")
            # 1.0 everywhere (ones columns); v cols overwritten below
            nc.gpsimd.memset(vsb.bitcast(mybir.dt.uint32), 0x3F800000)
            attnT = big.tile([128, 2, N], F32R, tag="attnT")

            # preamble psum evacuations alternate between DVE and the (still
            # idle) ScalarE so neither queue gates slot turnover
            _cp = [0]

            def copy_alt(dst, src_ap):
                _cp[0] += 1
                if _cp[0] % 2:
                    nc.vector.tensor_copy(dst, src_ap)
                else:
                    nc.scalar.copy(dst, src_ap)

            # ---- transpose x into padded x^T, conv interleaved ----
            def emit_transpose(nt):
                tp = pst.tile([128, 1024], F32, tag="ps", name="tp")
                tpr = tp.bitcast(F32R)
                for ct in range(2):
                    nc.tensor.transpose(
                        tpr[:, 512 * ct: 512 * ct + 128],
                        xins[nt][:, 128 * ct: 128 * (ct + 1)],
                        id_sb,
                    )
                    dst = xpadT[:, ct, :].rearrange("p (h w) -> p h w", h=PAD)[
                        :, 1 + 4 * nt: 5 + 4 * nt, 1:33
                    ]
                    copy_alt(
                        dst,
                        tp[:, 512 * ct: 512 * ct + 128].rearrange(
                            "p (a b) -> p a b", a=4
                        ),
                    )

            # conv accumulators: ct0 in the aux psum slot, ct1 in the (still
            # idle) PV slot, so transposes keep both pst slots
            cacc = [paux.tile([128, 1024], F32, tag="aux", name="cacc0"),
                    ppv.tile([128, 1024], F32, tag="pv", name="cacc1")]

            def emit_conv_part(ct, j, t0, t1, bias=False):
                cps = cacc[ct]
                view = xpadT[:, ct, :].rearrange("p (h w) -> p h w", h=PAD)
                for t in range(t0, t1):
                    ky, kx = TAPS[t]
                    nc.tensor.matmul(
                        cps[:, j * 512:(j + 1) * 512],
                        lhsT=diag_sb[:, ct * 9 + t, :],
                        rhs=view[:, ky + 16 * j: ky + 16 * j + 16, kx: kx + 32],
                        start=(t == 0),
                        stop=False,
                    )
                if bias:
                    nc.tensor.matmul(
                        cps[:, j * 512:(j + 1) * 512],
                        lhsT=convb_sb[0:1, 128 * ct: 128 * (ct + 1)],
                        rhs=ones_sb[0:1, j * 512:(j + 1) * 512],
                        start=False,
                        stop=True,
                    )

            def emit_conv_half(ct, j):
                emit_conv_part(ct, j, 0, 9, bias=True)

            # conv j=0 only needs padded rows 0..18 (x tiles 0..4) and
            # transposes 5..7 only feed conv j=1, so the preamble critical
            # path is transposes 0-4 -> conv(ct,0) -> q/k j=0.  conv j=1
            # (query tokens 512..1023, first needed at m-step 4 of pair 0)
            # is interleaved into pair 0's m-loop below.  The q/k j=0
            # contraction is split by feature chunk: the ct0 matmuls run
            # while conv ct1 still waits for its diag DMA.
            for nt in range(5):
                emit_transpose(nt)
            emit_conv_half(0, 0)
            nc.vector.tensor_copy(yT[:, 0, 0:512], cacc[0][:, 0:512])
            qk_ps = {}
            for ft in (0, 2):
                qk_ps[ft] = pst.tile([128, 1024], F32, tag="ps", name="qkps")
                nc.tensor.matmul(
                    qk_ps[ft][:, 0:512],
                    lhsT=qkvwT_sb[:, 0, (0 if ft < 2 else 256): (0 if ft < 2 else 256) + 128],
                    rhs=yT[:, 0, 0:512],
                    start=True,
                    stop=False,
                )
            emit_conv_half(1, 0)
            nc.scalar.copy(yT[:, 1, 0:512], cacc[1][:, 0:512])
            for ft, dstT in ((0, qT), (2, kT)):
                nc.tensor.matmul(
                    qk_ps[ft][:, 0:512],
                    lhsT=qkvwT_sb[:, 1, (0 if ft < 2 else 256): (0 if ft < 2 else 256) + 128],
                    rhs=yT[:, 1, 0:512],
                    start=False,
                    stop=True,
                )
                if ft == 0:
                    nc.scalar.copy(dstT[:, 0, 0:512], qk_ps[ft][:, 0:512])
                else:
                    nc.vector.tensor_copy(dstT[:, 0, 0:512], qk_ps[ft][:, 0:512])
            # transposes 5-7 only feed conv j=1 (pair-0 aux), off the
            # first-exp critical path
            for nt in range(5, 8):
                emit_transpose(nt)

            # ---- q^T / k^T feature tiles.  Tiles 0,2 (heads 0-3): the j=0
            # token halves run in the preamble, j=1 inside pair 0's m-loop.
            # Tiles 1,3 (heads 4-7) run inside pair 1's m-loop. ----
            def emit_qk(ft, pool, js=(0, 1), eng=None):
                dstT, dc = (qT, ft) if ft < 2 else (kT, ft - 2)
                fofs = 0 if ft < 2 else 256
                qps = pool.tile([128, 1024], F32, tag="ps" if pool is pst else "aux",
                                name="qps")
                for j in js:
                    for kc in range(2):
                        nc.tensor.matmul(
                            qps[:, j * 512:(j + 1) * 512],
                            lhsT=qkvwT_sb[:, kc, fofs + dc * 128: fofs + (dc + 1) * 128],
                            rhs=yT[:, kc, j * 512:(j + 1) * 512],
                            start=(kc == 0),
                            stop=(kc == 1),
                        )
                for j in js:
                    (eng or nc.vector).tensor_copy(
                        dstT[:, dc, j * 512:(j + 1) * 512],
                        qps[:, j * 512:(j + 1) * 512],
                    )

            def emit_v(nt):
                vps = paux.tile([128, 1024], F32, tag="aux", name="vps")
                for kc in range(2):
                    nc.tensor.matmul(
                        vps[:, 0:256],
                        lhsT=yT[:, kc, nt * 128:(nt + 1) * 128],
                        rhs=qkvwT_sb[:, kc, 512:768],
                        start=(kc == 0),
                        stop=(kc == 1),
                    )
                vv = vsb[:, nt, :].rearrange("p (hh c) -> p hh c", c=33)
                sv = vps[:, 0:256].rearrange("p (hh c) -> p hh c", c=32)
                nc.vector.tensor_copy(vv[:, :, 0:32], sv)  # [v_h | 1] per head

            emit_qk(0, pst, js=(0,))
            emit_qk(2, pst, js=(0,))

            # ---- out-projection of one 128-token tile (all 8 heads) ----
            _oq = [nc.sync, nc.scalar, nc.sync, nc.scalar]

            def emit_proj(nt, pool=None):
                pool = pool or paux
                ops = pool.tile([128, 1024], F32,
                                tag="ps" if pool is pst else "aux", name="ops")
                for chunk in range(2):
                    nc.tensor.matmul(
                        ops[:, 0:256],
                        lhsT=attnT[:, chunk, nt * 128:(nt + 1) * 128],
                        rhs=outwT_sb[:, chunk, :],
                        start=(chunk == 0),
                        stop=False,
                    )
                nc.tensor.matmul(
                    ops[:, 0:256],
                    lhsT=ones_sb[0:1, 0:128],
                    rhs=outb_sb,
                    start=False,
                    stop=True,
                )
                osb = outs_p.tile([128, C], F32, tag="o")
                nc.vector.tensor_copy(osb, ops[:, 0:256])
                _oq[nt % 4].dma_start(out_d[nt * 128:(nt + 1) * 128, :], osb)

            # ---- attention: two query halves x four head pairs ----
            # aux work interleaved into the m-loops, a slice per m-step:
            #   (half0, pair0): conv j=1 + its yT evacs, q/k tile-0 j=1
            #                   slices, v(0..7) (v(m) must precede PV(m))
            #   (half0, pair1): qk tiles 1,3 (needed by pair 2 = heads 5,7)
            #   (half1, pair0/1): projection of half-0 tiles 0..3
            def pair_extra(half, ip, m):
                if half == 0 and ip == 0:
                    if m == 0:
                        emit_conv_part(0, 1, 0, 5)
                    elif m == 1:
                        emit_conv_part(0, 1, 5, 9, bias=True)
                        nc.vector.tensor_copy(yT[:, 0, 512:1024],
                                              cacc[0][:, 512:1024])
                    elif m == 2:
                        emit_conv_part(1, 1, 0, 5)
                        emit_v(0)
                    elif m == 3:
                        emit_conv_part(1, 1, 5, 9, bias=True)
                        nc.vector.tensor_copy(yT[:, 1, 512:1024],
                                              cacc[1][:, 512:1024])
                        emit_qk(2, paux, js=(1,))
                        emit_v(1)
                    elif m == 4:
                        emit_v(2)
                    elif m == 5:
                        emit_v(3)
                        emit_v(4)
                    elif m == 6:
                        emit_v(5)
                        emit_v(6)
                    elif m == 7:
                        emit_v(7)
                        emit_qk(0, paux, js=(1,))
                elif half == 0 and ip == 1:
                    if m == 1:
                        emit_qk(1, paux)
                    elif m == 5:
                        emit_qk(3, paux)
                elif half == 1 and ip == 0 and m in (5, 7):
                    emit_proj((m - 5) // 2)
                elif half == 1 and ip == 1 and m in (1, 5):
                    emit_proj(2 + (m - 1) // 4)
                elif half == 1 and ip == 3 and m >= 4:
                    # partial projection of half-1 tiles: contributions from
                    # attn^T rows 32:64 and 96:128 (normalized well before
                    # this last pair -- rows 64:96 depend on pair (2,6)'s
                    # repositioning DMAs and join in the tail) plus the bias,
                    # staged to SBUF.
                    opsp = paux.tile([128, 1024], F32, tag="aux", name="opsp")
                    for chunk in range(2):
                        for a, k in ((32, 32), (64, 64)):
                            nc.tensor.matmul(
                                opsp[:, 0:256],
                                lhsT=attnT[a:a + k, chunk, m * 128:(m + 1) * 128],
                                rhs=outwT_sb[a:a + k, chunk, :],
                                start=(chunk == 0 and a == 32),
                                stop=False,
                                tile_position=(a, 0),
                            )
                    nc.tensor.matmul(
                        opsp[:, 0:256],
                        lhsT=ones_sb[0:1, 0:128],
                        rhs=outb_sb,
                        start=False,
                        stop=True,
                    )
                    nc.vector.tensor_copy(partial4[:, m - 4, :], opsp[:, 0:256])

            def emit_norm_b(half, hA, hB, pc, pool=None):
                # PE-broadcast the sums row (pc row 32) to 32 partitions,
                # reciprocal straight from psum, then normalize.
                pool = pool or paux
                bcp = pool.tile([128, 1024], F32,
                                tag="ps" if pool is pst else "aux", name="bcp")
                for j in range(2):
                    nc.tensor.matmul(
                        bcp[0:32, j * 512:(j + 1) * 512],
                        lhsT=onesp_sb[32:33, :],
                        rhs=pc[32:33, j * 512:(j + 1) * 512],
                        start=True,
                        stop=True,
                    )
                rs = rs_p.tile([128, 1024], F32, tag="rs")
                nc.vector.reciprocal(rs[0:32, :], bcp[0:32, :])
                pcb = pc.bitcast(F32)
                nofs = half * HALF
                for hd, h in ((0, hA), (1, hB)):
                    row = 32 * (h % 4)
                    hc = h // 4
                    cofs = hd * 512
                    if row == 0:
                        nc.vector.tensor_mul(
                            attnT[0:32, hc, nofs:nofs + HALF],
                            pcb[0:32, cofs:cofs + 512],
                            rs[0:32, cofs:cofs + 512],
                        )
                    else:
                        # reposition to the head's attn^T rows (DMA can shift
                        # partitions; DVE cannot)
                        pcs = tmp2_p.tile([128, 512], F32R, tag="pcs", name="pcs")
                        nc.vector.tensor_mul(
                            pcs[0:32, :],
                            pcb[0:32, cofs:cofs + 512],
                            rs[0:32, cofs:cofs + 512],
                        )
                        nc.sync.dma_start(
                            attnT[row:row + 32, hc, nofs:nofs + HALF], pcs[0:32, :]
                        )

            # The PV lag list is carried ACROSS pair boundaries: the trailing
            # PV accumulations of pair p are emitted during the first m-steps
            # of pair p+1, so the ScalarE exp stream never waits on a burst
            # of trailing PVs.  pv psum tiles are allocated lazily (at the
            # m==0 PV) so pair 0's conv-j1 accumulators can share the slots;
            # pair 0 uses lag 3 since v(0) only exists from its m-step 2.
            # The pv evacuation (one DVE copy) follows the m==7 PV; the rest
            # of the normalization lands >= m-step 3 of the next pair, well
            # past any PE dependency.
            pend = []  # (pid, half, hA, hB, m, pT) awaiting PV matmuls
            pending_norm = None  # (half, hA, hB, pc)
            pvt = {}  # pid -> lazily allocated pv psum tile
            partial4 = big.tile([128, 4, C], F32R, tag="partial4")

            def emit_pv(pid, half, hA, hB, m, pT):
                nonlocal pending_norm
                if m == 0:
                    pvt[pid] = ppv.tile([128, 1024], F32, tag="pv", name="pv")
                pv = pvt[pid]
                for hd, h in ((0, hA), (1, hB)):
                    nc.tensor.matmul(
                        pv[0:33, hd * 512:(hd + 1) * 512],
                        lhsT=vsb[:, m, 33 * h: 33 * h + 33],
                        rhs=pT[:, hd * 512:(hd + 1) * 512],
                        start=(m == 0),
                        stop=(m == 7),
                    )
                if m == 7:
                    pc = tmp_p.tile([128, 1024], F32R, tag="pc", name="pc")
                    nc.vector.tensor_copy(pc[0:33, :], pv[0:33, :])
                    pending_norm = (half, hA, hB, pc)
                    del pvt[pid]

            for half in range(2):
                for ip, (hA, hB) in enumerate(PAIRS_H[half]):
                    pid = half * 4 + ip
                    lag = 3 if pid == 0 else 2
                    for m in range(8):
                        st = pst.tile([128, 1024], F32, tag="ps")
                        for hd, h in ((0, hA), (1, hB)):
                            a = 32 * (h % 4)
                            hc = h // 4
                            nc.tensor.matmul(
                                st[:, hd * 512:(hd + 1) * 512],
                                lhsT=kT[a:a + 32, hc, m * 128:(m + 1) * 128],
                                rhs=qT[a:a + 32, hc, half * HALF: half * HALF + 512],
                                start=True,
                                stop=True,
                                tile_position=(a, 0),
                            )
                        # drain up to two prev-pair PVs per step (so the pv
                        # evacuation lands >= 2 steps before this pair's m==0
                        # PV reuses the psum slot), plus one own-pair PV once
                        # past the lag
                        for _ in range(2):
                            if pend and pend[0][0] != pid:
                                emit_pv(*pend.pop(0))
                        if pend and pend[0][0] == pid and len(pend) > lag:
                            emit_pv(*pend.pop(0))
                        if pending_norm is not None and m >= 2:
                            emit_norm_b(*pending_norm)
                            pending_norm = None
                        pT = ppool.tile([128, 1024], F32R, tag="pT")
                        nc.scalar.activation(pT, st, AF.Exp, bias=zerob_sb, scale=SCALE)
                        pair_extra(half, ip, m)
                        pend.append((pid, half, hA, hB, m, pT))
            # ---- tail: drain the PV backlog; the last pair's (m==7) PVs are
            # followed per-head by a short normalize chain (copy, broadcast,
            # reciprocal, multiply -- heads 0 and 4 both land on rows 0:32 so
            # there is no repositioning), then the half-1 projection finishes
            # with the two K=32 row-0 contributions per tile and stores.
            while len(pend) > 1:
                emit_pv(*pend.pop(0))
            lpid, lhalf, lhA, lhB, lm, lpT = pend.pop(0)
            pv = pvt[lpid]
            nofs = lhalf * HALF
            for hd, h in ((0, lhA), (1, lhB)):
                nc.tensor.matmul(
                    pv[0:33, hd * 512:(hd + 1) * 512],
                    lhsT=vsb[:, lm, 33 * h: 33 * h + 33],
                    rhs=lpT[:, hd * 512:(hd + 1) * 512],
                    start=False,
                    stop=True,
                )
            # evacuate the two heads' unnormalized tiles + sums, one on the
            # (now idle) ScalarE and one on DVE so they overlap
            pcH = [None, None]
            for hd, h in ((0, lhA), (1, lhB)):
                pcH[hd] = tmp2_p.tile([128, 512], F32R, tag="pcs", name="pcH")
                if hd == 0:
                    nc.scalar.copy(pcH[hd][0:33, :],
                                   pv[0:33, hd * 512:(hd + 1) * 512])
                else:
                    nc.vector.tensor_copy(pcH[hd][0:33, :],
                                          pv[0:33, hd * 512:(hd + 1) * 512])
            # open the final psum groups: rows 64:96 (now safely landed from
            # pair (2,6)'s repositioning) plus the staged partial via an
            # identity matmul -- all independent of this pair's normalize.
            # Two output tiles per [128,1024] psum tile, in separate banks.
            opsf = [pst.tile([128, 1024], F32, tag="ps", name="opsfA"),
                    paux.tile([128, 1024], F32, tag="aux", name="opsfB")]
            for i, nt in enumerate(range(4, 8)):
                sl = opsf[i // 2][:, (i % 2) * 512:(i % 2) * 512 + 256]
                nc.tensor.matmul(
                    sl,
                    lhsT=id_sb,
                    rhs=partial4[:, i, :],
                    start=True,
                    stop=False,
                )
            # per-head: broadcast sums, reciprocal, normalize (both heads of
            # the last pair land on rows 0:32 -- no repositioning)
            bcp2 = pst.tile([128, 1024], F32, tag="ps", name="bcp2")
            rsH = [None, None]
            for hd, h in ((0, lhA), (1, lhB)):
                nc.tensor.matmul(
                    bcp2[0:32, hd * 512:(hd + 1) * 512],
                    lhsT=onesp_sb[32:33, :],
                    rhs=pcH[hd][32:33, :],
                    start=True,
                    stop=True,
                )
                rsH[hd] = rs_p.tile([128, 512], F32, tag="rs", name="rsH")
                nc.vector.reciprocal(rsH[hd][0:32, :],
                                     bcp2[0:32, hd * 512:(hd + 1) * 512])
                nc.vector.tensor_mul(
                    attnT[0:32, h // 4, nofs:nofs + HALF],
                    pcH[hd].bitcast(F32)[0:32, :],
                    rsH[hd][0:32, :],
                )

            if debug_dump:
                nc.sync.dma_start(dbg["d_yT"], yT.bitcast(F32))
                nc.sync.dma_start(dbg["d_qT"], qT.bitcast(F32))
                nc.sync.dma_start(dbg["d_kT"], kT.bitcast(F32))
                nc.sync.dma_start(dbg["d_v"], vsb.bitcast(F32))
                nc.sync.dma_start(dbg["d_attnT"], attnT.bitcast(F32))

            # final row-0 contributions (all chunk-0 matmuls as soon as head
            # lhA's normalize lands, chunk-1 after lhB's), then one paired
            # evacuation + store per psum tile
            for chunk in range(2):
                for i, nt in enumerate(range(4, 8)):
                    nc.tensor.matmul(
                        opsf[i // 2][:, (i % 2) * 512:(i % 2) * 512 + 256],
                        lhsT=attnT[0:32, chunk, nt * 128:(nt + 1) * 128],
                        rhs=outwT_sb[0:32, chunk, :],
                        start=False,
                        stop=(chunk == 1),
                        tile_position=(0, 0),
                    )
            for half_t, eng, q in ((0, nc.vector, nc.sync), (1, None, nc.scalar)):
                osb2 = outs_p.tile([128, 2, C], F32, tag="o", name="osb2")
                src = opsf[half_t].rearrange("p (t c) -> p t c", t=2)[:, :, 0:256]
                if eng is None:
                    nc.scalar.copy(osb2, src)
                else:
                    eng.tensor_copy(osb2, src)
                r0 = 512 + half_t * 256
                q.dma_start(
                    out_d[r0:r0 + 256, :].rearrange("(t p) c -> p t c", p=128),
                    osb2,
                )

    nc.compile()
    return nc


_NC = None
LAST_RESULTS = None


def _host_prep(conv_w, conv_b, qkv_w, out_w, out_b):
    conv_w = np.asarray(conv_w, np.float32).reshape(C, 3, 3)
    diag = np.zeros((2, 9, 128, 128), np.float32)
    idx = np.arange(128)
    for ct in range(2):
        for t, (ky, kx) in enumerate(TAPS):
            d = conv_w[128 * ct: 128 * (ct + 1), ky, kx].copy()
            if (ky, kx) == (1, 1):
                d += 1.0  # residual connection folded into the center tap
            diag[ct, t, idx, idx] = d
    return {
        "qkv_wT": np.ascontiguousarray(np.asarray(qkv_w, np.float32).T),
        "out_wT": np.ascontiguousarray(np.asarray(out_w, np.float32).T),
        "conv_diag": diag,
        "conv_b_r": np.asarray(conv_b, np.float32).reshape(1, C),
        "out_b_r": np.asarray(out_b, np.float32).reshape(1, C),
        "id128": np.eye(128, dtype=np.float32),
    }


def kernel(x, conv_w, conv_b, qkv_w, out_w, out_b):
    global _NC, LAST_RESULTS
    if _NC is None:
        _NC = build_nc()
    x = np.asarray(x, np.float32)
    shared = _host_prep(conv_w, conv_b, qkv_w, out_w, out_b)
    in_maps = [{**shared, "x": np.ascontiguousarray(x[b])} for b in range(B)]
    trace = bool(int(os.environ.get("KERNEL_TRACE", "0")))
    try:
        res = run_bass_kernel_spmd(_NC, in_maps, core_ids=list(range(B)), trace=trace)
    except Exception:
        if not trace:
            raise
        res = run_bass_kernel_spmd(_NC, in_maps, core_ids=list(range(B)), trace=False)
    LAST_RESULTS = res
    return np.stack([res.results[b]["out"] for b in range(B)], axis=0)


# revision 51
# speedup vs baseline: 1.2297x; 1.0292x over previous
"""Trainium2 Bass kernel for nn_Attention_43190191129190.

Model (per batch element b of 8):
    y   = x + dwconv3x3(x) + conv_b          (depthwise residual positional conv)
    qkv = y @ qkv_w.T ; split into q, k, v   (8 heads, dim 32)
    out = softmax(q k^T / sqrt(32)) v
    out = out @ out_w.T + out_b

Sharding: pure data-parallel, one batch element per NeuronCore (8 cores).

Per-core design (transposed [C, N] space; see kernel_baseline.py.bak for the
ancestor).  v2 changes vs the baseline:

  * Attention is split over the QUERY dim into two 512-token halves.  All 4
    head pairs run on half 0, whose normalization + out-projection + store
    overlap half 1's attention; only half 1's last pair remains in the tail.
  * PSUM re-plan: pst 2x[128,1024] (S^T double-buffer), ppv 1x[128,1024]
    (PV accumulator, freed by a single DVE evacuation), paux 1x[128,1024]
    (conv ct0 / v / qk tiles 1,3 / softmax-broadcast / projection).
  * Softmax normalization with no DRAM round trip: evacuate pv -> pc (SBUF),
    PE-broadcast the sums row (ones stationary), DVE reciprocal from PSUM,
    DVE multiply; deferred past the next pair's first S^T so PE never waits.
  * ScalarE runs exps only (one [128,1024] exp per pair/m covering both
    heads); all other evacuations are on DVE (preamble ones may use ScalarE
    while it is still idle).
  * Input DMAs spread over the four HWDGE queues + the Pool SWDGE path.
"""

import os

import numpy as np

import concourse.bass as bass
import concourse.tile as tile
from concourse import bacc, mybir
from concourse.bass_utils import run_bass_kernel_spmd

F32 = mybir.dt.float32
F32R = mybir.dt.float32r
AF = mybir.ActivationFunctionType

B, N, C = 8, 1024, 256
HEADS, DH = 8, 32
SCALE = DH ** -0.5
PAD = 34  # 32x32 spatial grid with 1-px halo
HALF = 512

TAPS = [(ky, kx) for ky in range(3) for kx in range(3)]
# Head pairs per query half.  Half 0 must order so q/k feature tiles 1,3
# (emitted during its pair 1) precede any head >= 4.  Half 1 re-pairs so the
# final pair (0,4) writes attn^T rows 0:32 of both chunks directly -- no
# repositioning DMA in the kernel tail.
PAIRS_H = [
    [(1, 3), (0, 2), (5, 7), (4, 6)],
    [(1, 3), (5, 7), (2, 6), (0, 4)],
]


def build_nc(debug_dump=False):
    nc = bacc.Bacc("TRN2", target_bir_lowering=False, debug=False, num_devices=8)

    x_d = nc.dram_tensor("x", (N, C), F32R, kind="ExternalInput").ap()
    qkvwT_d = nc.dram_tensor("qkv_wT", (C, 3 * C), F32R, kind="ExternalInput").ap()
    outwT_d = nc.dram_tensor("out_wT", (C, C), F32R, kind="ExternalInput").ap()
    diag_d = nc.dram_tensor("conv_diag", (2, 9, 128, 128), F32R, kind="ExternalInput").ap()
    convb_d = nc.dram_tensor("conv_b_r", (1, C), F32R, kind="ExternalInput").ap()
    outb_d = nc.dram_tensor("out_b_r", (1, C), F32R, kind="ExternalInput").ap()
    id_d = nc.dram_tensor("id128", (128, 128), F32R, kind="ExternalInput").ap()
    out_d = nc.dram_tensor("out", (N, C), F32, kind="ExternalOutput").ap()
    dbg = {}
    if debug_dump:
        for name, shape in (
            ("d_yT", (128, 2, N)), ("d_qT", (128, 2, N)), ("d_kT", (128, 2, N)),
            ("d_v", (128, 8, 8 * 33)), ("d_attnT", (128, 2, N)),
        ):
            dbg[name] = nc.dram_tensor(name, shape, F32, kind="ExternalOutput").ap()

    with tile.TileContext(nc) as tc:
        with (
            tc.tile_pool(name="const", bufs=1) as const,
            tc.tile_pool(name="xin", bufs=1) as xin_p,
            tc.tile_pool(name="big", bufs=1) as big,
            tc.tile_pool(name="pT", bufs=8) as ppool,
            tc.tile_pool(name="rs", bufs=2) as rs_p,
            tc.tile_pool(name="tmp", bufs=2) as tmp_p,
            tc.tile_pool(name="tmp2", bufs=2) as tmp2_p,
            tc.tile_pool(name="outs", bufs=3) as outs_p,
            tc.tile_pool(name="pst", bufs=2, space="PSUM") as pst,
            tc.tile_pool(name="ppv", bufs=1, space="PSUM") as ppv,
            tc.tile_pool(name="paux", bufs=1, space="PSUM") as paux,
        ):
            # ---- input DMAs.  Critical path: id128 + x0..x4 + diag(ct0)
            # (transposes then conv j=0).  HWDGE is a single shared device
            # (~630ns serialization per DMA), so id128 and the noncritical
            # loads go via the parallel Pool SWDGE path; x0..x4 + the conv
            # diag lead the HWDGE order, weights follow.
            id_sb = const.tile([128, 128], F32R, tag="id")
            nc.gpsimd.dma_start(id_sb, id_d)
            xins = [None] * 8
            _xq = {0: nc.sync, 1: nc.scalar, 2: nc.sync, 3: nc.scalar,
                   4: nc.scalar, 5: nc.gpsimd, 6: nc.gpsimd, 7: nc.gpsimd}
            diag_sb = const.tile([128, 18, 128], F32R, tag="diag")
            diag_r = diag_d.rearrange("ct t p f -> p (ct t) f")

            def load_x(nt):
                xin = xin_p.tile([128, C], F32R, tag=f"xin{nt}", name=f"xin{nt}")
                _xq[nt].dma_start(xin, x_d[nt * 128:(nt + 1) * 128, :])
                xins[nt] = xin

            # HWDGE DGEs serialize in emission order (~630ns apiece) and the
            # DMA-engine pool drains transfers roughly in DGE-completion
            # order, so the emission order below IS the arrival order.  The
            # conv diag is split in four so each chunk lands just before its
            # taps run; q/k weights (first 512 features) precede v weights.
            qkvwT_sb = const.tile([128, 2, 3 * C], F32R, tag="qkvwT")
            qkvw_r = qkvwT_d.rearrange("(kc p) f -> p kc f", p=128)
            for nt in (0, 1, 2, 3):
                load_x(nt)
            nc.sync.dma_start(diag_sb[:, 0:5, :], diag_r[:, 0:5, :])
            load_x(4)
            nc.sync.dma_start(diag_sb[:, 5:9, :], diag_r[:, 5:9, :])
            nc.scalar.dma_start(diag_sb[:, 9:14, :], diag_r[:, 9:14, :])
            nc.sync.dma_start(qkvwT_sb[:, :, 0:512], qkvw_r[:, :, 0:512])
            nc.scalar.dma_start(diag_sb[:, 14:18, :], diag_r[:, 14:18, :])
            nc.sync.dma_start(qkvwT_sb[:, :, 512:768], qkvw_r[:, :, 512:768])
            for nt in (5, 6, 7):
                load_x(nt)
            convb_sb = const.tile([1, C], F32R, tag="convb")
            nc.gpsimd.dma_start(convb_sb, convb_d)
            ones_sb = const.tile([1, N], F32R, tag="ones")
            nc.gpsimd.memset(ones_sb.bitcast(mybir.dt.uint32), 0x3F800000)
            outwT_sb = const.tile([128, 2, C], F32R, tag="outwT")
            nc.gpsimd.dma_start(outwT_sb, outwT_d.rearrange("(kc p) f -> p kc f", p=128))
            outb_sb = const.tile([1, C], F32R, tag="outb")
            nc.gpsimd.dma_start(outb_sb, outb_d)
            zerob_sb = const.tile([128, 1], F32, tag="zerob")
            nc.vector.memset(zerob_sb, 0.0)
            # dummy exp: hoists the ~1.3us exp_and_others ACT table load into
            # the idle startup window (the set also contains Copy, so the
            # preamble ScalarE copies share it)
            warm_sb = const.tile([1, 1], F32, tag="warm")
            nc.scalar.activation(
                warm_sb, zerob_sb[0:1, 0:1], AF.Exp,
                bias=zerob_sb[0:1], scale=1.0,
            )
            # all-ones strip on every partition (PE broadcast stationary must
            # share its base partition with the moving operand)
            onesp_sb = const.tile([128, 32], F32R, tag="onesp")
            nc.gpsimd.memset(onesp_sb.bitcast(mybir.dt.uint32), 0x3F800000)

            # ---- persistent activations ----
            xpadT = big.tile([128, 2, PAD * PAD], F32R, tag="xpadT")
            # zero only the 1-px halo ring (interior is fully overwritten)
            xpv = xpadT.bitcast(mybir.dt.uint32).rearrange(
                "p ct (h w) -> p ct h w", h=PAD
            )
            nc.gpsimd.memset(xpv[:, :, 0, :], 0)
            nc.gpsimd.memset(xpv[:, :, PAD - 1, :], 0)
            nc.gpsimd.memset(xpv[:, :, :, 0], 0)
            nc.gpsimd.memset(xpv[:, :, :, PAD - 1], 0)
            yT = big.tile([128, 2, N], F32R, tag="yT")
            qT = big.tile([128, 2, N], F32R, tag="qT")
            kT = big.tile([128, 2, N], F32R, tag="kT")
            vsb = big.tile([128, 8, 8 * 33], F32R, tag="v")
            # 1.0 everywhere (ones columns); v cols overwritten below
            nc.gpsimd.memset(vsb.bitcast(mybir.dt.uint32), 0x3F800000)
            attnT = big.tile([128, 2, N], F32R, tag="attnT")

            # preamble psum evacuations alternate between DVE and the (still
            # idle) ScalarE so neither queue gates slot turnover
            _cp = [0]

            def copy_alt(dst, src_ap):
                _cp[0] += 1
                if _cp[0] % 2:
                    nc.vector.tensor_copy(dst, src_ap)
                else:
                    nc.scalar.copy(dst, src_ap)

            # ---- transpose x into padded x^T, conv interleaved ----
            def emit_transpose(nt):
                tp = pst.tile([128, 1024], F32, tag="ps", name="tp")
                tpr = tp.bitcast(F32R)
                for ct in range(2):
                    nc.tensor.transpose(
                        tpr[:, 512 * ct: 512 * ct + 128],
                        xins[nt][:, 128 * ct: 128 * (ct + 1)],
                        id_sb,
                    )
                    dst = xpadT[:, ct, :].rearrange("p (h w) -> p h w", h=PAD)[
                        :, 1 + 4 * nt: 5 + 4 * nt, 1:33
                    ]
                    copy_alt(
                        dst,
                        tp[:, 512 * ct: 512 * ct + 128].rearrange(
                            "p (a b) -> p a b", a=4
                        ),
                    )

            # conv accumulators: ct0 in the aux psum slot, ct1 in the (still
            # idle) PV slot, so transposes keep both pst slots
            cacc = [paux.tile([128, 1024], F32, tag="aux", name="cacc0"),
                    ppv.tile([128, 1024], F32, tag="pv", name="cacc1")]

            def emit_conv_part(ct, j, t0, t1, bias=False):
                cps = cacc[ct]
                view = xpadT[:, ct, :].rearrange("p (h w) -> p h w", h=PAD)
                for t in range(t0, t1):
                    ky, kx = TAPS[t]
                    nc.tensor.matmul(
                        cps[:, j * 512:(j + 1) * 512],
                        lhsT=diag_sb[:, ct * 9 + t, :],
                        rhs=view[:, ky + 16 * j: ky + 16 * j + 16, kx: kx + 32],
                        start=(t == 0),
                        stop=False,
                    )
                if bias:
                    nc.tensor.matmul(
                        cps[:, j * 512:(j + 1) * 512],
                        lhsT=convb_sb[0:1, 128 * ct: 128 * (ct + 1)],
                        rhs=ones_sb[0:1, j * 512:(j + 1) * 512],
                        start=False,
                        stop=True,
                    )

            def emit_conv_half(ct, j):
                emit_conv_part(ct, j, 0, 9, bias=True)

            # conv j=0 only needs padded rows 0..18 (x tiles 0..4) and
            # transposes 5..7 only feed conv j=1, so the preamble critical
            # path is transposes 0-4 -> conv(ct,0) -> q/k j=0.  conv j=1
            # (query tokens 512..1023, first needed at m-step 4 of pair 0)
            # is interleaved into pair 0's m-loop below.  The q/k j=0
            # contraction is split by feature chunk: the ct0 matmuls run
            # while conv ct1 still waits for its diag DMA.
            for nt in range(5):
                emit_transpose(nt)
            emit_conv_half(0, 0)
            nc.vector.tensor_copy(yT[:, 0, 0:512], cacc[0][:, 0:512])
            for nt in range(5, 8):
                emit_transpose(nt)
            qk_ps = {}
            for ft in (0, 2):
                qk_ps[ft] = pst.tile([128, 1024], F32, tag="ps", name="qkps")
                nc.tensor.matmul(
                    qk_ps[ft][:, 0:512],
                    lhsT=qkvwT_sb[:, 0, (0 if ft < 2 else 256): (0 if ft < 2 else 256) + 128],
                    rhs=yT[:, 0, 0:512],
                    start=True,
                    stop=False,
                )
            emit_conv_half(1, 0)
            nc.scalar.copy(yT[:, 1, 0:512], cacc[1][:, 0:512])
            for ft, dstT in ((0, qT), (2, kT)):
                nc.tensor.matmul(
                    qk_ps[ft][:, 0:512],
                    lhsT=qkvwT_sb[:, 1, (0 if ft < 2 else 256): (0 if ft < 2 else 256) + 128],
                    rhs=yT[:, 1, 0:512],
                    start=False,
                    stop=True,
                )
                nc.vector.tensor_copy(dstT[:, 0, 0:512], qk_ps[ft][:, 0:512])

            # ---- q^T / k^T feature tiles.  Tiles 0,2 (heads 0-3): the j=0
            # token halves run in the preamble, j=1 inside pair 0's m-loop.
            # Tiles 1,3 (heads 4-7) run inside pair 1's m-loop. ----
            def emit_qk(ft, pool, js=(0, 1), eng=None):
                dstT, dc = (qT, ft) if ft < 2 else (kT, ft - 2)
                fofs = 0 if ft < 2 else 256
                qps = pool.tile([128, 1024], F32, tag="ps" if pool is pst else "aux",
                                name="qps")
                for j in js:
                    for kc in range(2):
                        nc.tensor.matmul(
                            qps[:, j * 512:(j + 1) * 512],
                            lhsT=qkvwT_sb[:, kc, fofs + dc * 128: fofs + (dc + 1) * 128],
                            rhs=yT[:, kc, j * 512:(j + 1) * 512],
                            start=(kc == 0),
                            stop=(kc == 1),
                        )
                for j in js:
                    (eng or nc.vector).tensor_copy(
                        dstT[:, dc, j * 512:(j + 1) * 512],
                        qps[:, j * 512:(j + 1) * 512],
                    )

            def emit_v(nt):
                vps = paux.tile([128, 1024], F32, tag="aux", name="vps")
                for kc in range(2):
                    nc.tensor.matmul(
                        vps[:, 0:256],
                        lhsT=yT[:, kc, nt * 128:(nt + 1) * 128],
                        rhs=qkvwT_sb[:, kc, 512:768],
                        start=(kc == 0),
                        stop=(kc == 1),
                    )
                vv = vsb[:, nt, :].rearrange("p (hh c) -> p hh c", c=33)
                sv = vps[:, 0:256].rearrange("p (hh c) -> p hh c", c=32)
                nc.vector.tensor_copy(vv[:, :, 0:32], sv)  # [v_h | 1] per head

            emit_qk(0, pst, js=(0,))
            emit_qk(2, pst, js=(0,))

            # ---- out-projection of one 128-token tile (all 8 heads) ----
            _oq = [nc.sync, nc.scalar, nc.sync, nc.scalar]

            def emit_proj(nt, pool=None):
                pool = pool or paux
                ops = pool.tile([128, 1024], F32,
                                tag="ps" if pool is pst else "aux", name="ops")
                for chunk in range(2):
                    nc.tensor.matmul(
                        ops[:, 0:256],
                        lhsT=attnT[:, chunk, nt * 128:(nt + 1) * 128],
                        rhs=outwT_sb[:, chunk, :],
                        start=(chunk == 0),
                        stop=False,
                    )
                nc.tensor.matmul(
                    ops[:, 0:256],
                    lhsT=ones_sb[0:1, 0:128],
                    rhs=outb_sb,
                    start=False,
                    stop=True,
                )
                osb = outs_p.tile([128, C], F32, tag="o")
                nc.vector.tensor_copy(osb, ops[:, 0:256])
                _oq[nt % 4].dma_start(out_d[nt * 128:(nt + 1) * 128, :], osb)

            # ---- attention: two query halves x four head pairs ----
            # aux work interleaved into the m-loops, a slice per m-step:
            #   (half0, pair0): conv j=1 + its yT evacs, q/k tile-0 j=1
            #                   slices, v(0..7) (v(m) must precede PV(m))
            #   (half0, pair1): qk tiles 1,3 (needed by pair 2 = heads 5,7)
            #   (half1, pair0/1): projection of half-0 tiles 0..3
            def pair_extra(half, ip, m):
                if half == 0 and ip == 0:
                    if m == 0:
                        emit_conv_part(0, 1, 0, 5)
                    elif m == 1:
                        emit_conv_part(0, 1, 5, 9, bias=True)
                        nc.vector.tensor_copy(yT[:, 0, 512:1024],
                                              cacc[0][:, 512:1024])
                    elif m == 2:
                        emit_conv_part(1, 1, 0, 5)
                        emit_v(0)
                    elif m == 3:
                        emit_conv_part(1, 1, 5, 9, bias=True)
                        nc.vector.tensor_copy(yT[:, 1, 512:1024],
                                              cacc[1][:, 512:1024])
                        emit_qk(2, paux, js=(1,))
                        emit_v(1)
                    elif m == 4:
                        emit_v(2)
                    elif m == 5:
                        emit_v(3)
                        emit_v(4)
                    elif m == 6:
                        emit_v(5)
                        emit_v(6)
                    elif m == 7:
                        emit_v(7)
                        emit_qk(0, paux, js=(1,))
                elif half == 0 and ip == 1:
                    if m == 1:
                        emit_qk(1, paux)
                    elif m == 5:
                        emit_qk(3, paux)
                elif half == 1 and ip == 0 and m in (5, 7):
                    emit_proj((m - 5) // 2)
                elif half == 1 and ip == 1 and m in (1, 5):
                    emit_proj(2 + (m - 1) // 4)
                elif half == 1 and ip == 3 and m >= 4:
                    # partial projection of half-1 tiles: contributions from
                    # attn^T rows 32:64 and 96:128 (normalized well before
                    # this last pair -- rows 64:96 depend on pair (2,6)'s
                    # repositioning DMAs and join in the tail) plus the bias,
                    # staged to SBUF.
                    opsp = paux.tile([128, 1024], F32, tag="aux", name="opsp")
                    for chunk in range(2):
                        for a, k in ((32, 32), (64, 64)):
                            nc.tensor.matmul(
                                opsp[:, 0:256],
                                lhsT=attnT[a:a + k, chunk, m * 128:(m + 1) * 128],
                                rhs=outwT_sb[a:a + k, chunk, :],
                                start=(chunk == 0 and a == 32),
                                stop=False,
                                tile_position=(a, 0),
                            )
                    nc.tensor.matmul(
                        opsp[:, 0:256],
                        lhsT=ones_sb[0:1, 0:128],
                        rhs=outb_sb,
                        start=False,
                        stop=True,
                    )
                    nc.vector.tensor_copy(partial4[:, m - 4, :], opsp[:, 0:256])

            def emit_norm_b(half, hA, hB, pc, pool=None):
                # PE-broadcast the sums row (pc row 32) to 32 partitions,
                # reciprocal straight from psum, then normalize.
                pool = pool or paux
                bcp = pool.tile([128, 1024], F32,
                                tag="ps" if pool is pst else "aux", name="bcp")
                for j in range(2):
                    nc.tensor.matmul(
                        bcp[0:32, j * 512:(j + 1) * 512],
                        lhsT=onesp_sb[32:33, :],
                        rhs=pc[32:33, j * 512:(j + 1) * 512],
                        start=True,
                        stop=True,
                    )
                rs = rs_p.tile([128, 1024], F32, tag="rs")
                nc.vector.reciprocal(rs[0:32, :], bcp[0:32, :])
                pcb = pc.bitcast(F32)
                nofs = half * HALF
                for hd, h in ((0, hA), (1, hB)):
                    row = 32 * (h % 4)
                    hc = h // 4
                    cofs = hd * 512
                    if row == 0:
                        nc.vector.tensor_mul(
                            attnT[0:32, hc, nofs:nofs + HALF],
                            pcb[0:32, cofs:cofs + 512],
                            rs[0:32, cofs:cofs + 512],
                        )
                    else:
                        # reposition to the head's attn^T rows (DMA can shift
                        # partitions; DVE cannot)
                        pcs = tmp2_p.tile([128, 512], F32R, tag="pcs", name="pcs")
                        nc.vector.tensor_mul(
                            pcs[0:32, :],
                            pcb[0:32, cofs:cofs + 512],
                            rs[0:32, cofs:cofs + 512],
                        )
                        nc.sync.dma_start(
                            attnT[row:row + 32, hc, nofs:nofs + HALF], pcs[0:32, :]
                        )

            # The PV lag list is carried ACROSS pair boundaries: the trailing
            # PV accumulations of pair p are emitted during the first m-steps
            # of pair p+1, so the ScalarE exp stream never waits on a burst
            # of trailing PVs.  pv psum tiles are allocated lazily (at the
            # m==0 PV) so pair 0's conv-j1 accumulators can share the slots;
            # pair 0 uses lag 3 since v(0) only exists from its m-step 2.
            # The pv evacuation (one DVE copy) follows the m==7 PV; the rest
            # of the normalization lands >= m-step 3 of the next pair, well
            # past any PE dependency.
            pend = []  # (pid, half, hA, hB, m, pT) awaiting PV matmuls
            pending_norm = None  # (half, hA, hB, pc)
            pvt = {}  # pid -> lazily allocated pv psum tile
            partial4 = big.tile([128, 4, C], F32R, tag="partial4")

            def emit_pv(pid, half, hA, hB, m, pT):
                nonlocal pending_norm
                if m == 0:
                    pvt[pid] = ppv.tile([128, 1024], F32, tag="pv", name="pv")
                pv = pvt[pid]
                for hd, h in ((0, hA), (1, hB)):
                    nc.tensor.matmul(
                        pv[0:33, hd * 512:(hd + 1) * 512],
                        lhsT=vsb[:, m, 33 * h: 33 * h + 33],
                        rhs=pT[:, hd * 512:(hd + 1) * 512],
                        start=(m == 0),
                        stop=(m == 7),
                    )
                if m == 7:
                    pc = tmp_p.tile([128, 1024], F32R, tag="pc", name="pc")
                    nc.vector.tensor_copy(pc[0:33, :], pv[0:33, :])
                    pending_norm = (half, hA, hB, pc)
                    del pvt[pid]

            for half in range(2):
                for ip, (hA, hB) in enumerate(PAIRS_H[half]):
                    pid = half * 4 + ip
                    lag = 3 if pid == 0 else 2
                    for m in range(8):
                        st = pst.tile([128, 1024], F32, tag="ps")
                        for hd, h in ((0, hA), (1, hB)):
                            a = 32 * (h % 4)
                            hc = h // 4
                            nc.tensor.matmul(
                                st[:, hd * 512:(hd + 1) * 512],
                                lhsT=kT[a:a + 32, hc, m * 128:(m + 1) * 128],
                                rhs=qT[a:a + 32, hc, half * HALF: half * HALF + 512],
                                start=True,
                                stop=True,
                                tile_position=(a, 0),
                            )
                        # drain up to two prev-pair PVs per step (so the pv
                        # evacuation lands >= 2 steps before this pair's m==0
                        # PV reuses the psum slot), plus one own-pair PV once
                        # past the lag
                        for _ in range(2):
                            if pend and pend[0][0] != pid:
                                emit_pv(*pend.pop(0))
                        if pend and pend[0][0] == pid and len(pend) > lag:
                            emit_pv(*pend.pop(0))
                        if pending_norm is not None and m >= 2:
                            emit_norm_b(*pending_norm)
                            pending_norm = None
                        pT = ppool.tile([128, 1024], F32R, tag="pT")
                        nc.scalar.activation(pT, st, AF.Exp, bias=zerob_sb, scale=SCALE)
                        pair_extra(half, ip, m)
                        pend.append((pid, half, hA, hB, m, pT))
            # ---- tail: drain the PV backlog; the last pair's (m==7) PVs are
            # followed per-head by a short normalize chain (copy, broadcast,
            # reciprocal, multiply -- heads 0 and 4 both land on rows 0:32 so
            # there is no repositioning), then the half-1 projection finishes
            # with the two K=32 row-0 contributions per tile and stores.
            while len(pend) > 1:
                emit_pv(*pend.pop(0))
            lpid, lhalf, lhA, lhB, lm, lpT = pend.pop(0)
            pv = pvt[lpid]
            nofs = lhalf * HALF
            for hd, h in ((0, lhA), (1, lhB)):
                nc.tensor.matmul(
                    pv[0:33, hd * 512:(hd + 1) * 512],
                    lhsT=vsb[:, lm, 33 * h: 33 * h + 33],
                    rhs=lpT[:, hd * 512:(hd + 1) * 512],
                    start=False,
                    stop=True,
                )
            # evacuate the two heads' unnormalized tiles + sums, one on the
            # (now idle) ScalarE and one on DVE so they overlap
            pcH = [None, None]
            for hd, h in ((0, lhA), (1, lhB)):
                pcH[hd] = tmp2_p.tile([128, 512], F32R, tag="pcs", name="pcH")
                if hd == 0:
                    nc.scalar.copy(pcH[hd][0:33, :],
                                   pv[0:33, hd * 512:(hd + 1) * 512])
                else:
                    nc.vector.tensor_copy(pcH[hd][0:33, :],
                                          pv[0:33, hd * 512:(hd + 1) * 512])
            # open the final psum groups: rows 64:96 (now safely landed from
            # pair (2,6)'s repositioning) plus the staged partial via an
            # identity matmul -- all independent of this pair's normalize.
            # Two output tiles per [128,1024] psum tile, in separate banks.
            opsf = [pst.tile([128, 1024], F32, tag="ps", name="opsfA"),
                    paux.tile([128, 1024], F32, tag="aux", name="opsfB")]
            for i, nt in enumerate(range(4, 8)):
                sl = opsf[i // 2][:, (i % 2) * 512:(i % 2) * 512 + 256]
                nc.tensor.matmul(
                    sl,
                    lhsT=id_sb,
                    rhs=partial4[:, i, :],
                    start=True,
                    stop=False,
                )
            # per-head: broadcast sums, reciprocal, normalize (both heads of
            # the last pair land on rows 0:32 -- no repositioning)
            bcp2 = pst.tile([128, 1024], F32, tag="ps", name="bcp2")
            rsH = [None, None]
            for hd, h in ((0, lhA), (1, lhB)):
                nc.tensor.matmul(
                    bcp2[0:32, hd * 512:(hd + 1) * 512],
                    lhsT=onesp_sb[32:33, :],
                    rhs=pcH[hd][32:33, :],
                    start=True,
                    stop=True,
                )
                rsH[hd] = rs_p.tile([128, 512], F32, tag="rs", name="rsH")
                nc.vector.reciprocal(rsH[hd][0:32, :],
                                     bcp2[0:32, hd * 512:(hd + 1) * 512])
                nc.vector.tensor_mul(
                    attnT[0:32, h // 4, nofs:nofs + HALF],
                    pcH[hd].bitcast(F32)[0:32, :],
                    rsH[hd][0:32, :],
                )

            if debug_dump:
                nc.sync.dma_start(dbg["d_yT"], yT.bitcast(F32))
                nc.sync.dma_start(dbg["d_qT"], qT.bitcast(F32))
                nc.sync.dma_start(dbg["d_kT"], kT.bitcast(F32))
                nc.sync.dma_start(dbg["d_v"], vsb.bitcast(F32))
                nc.sync.dma_start(dbg["d_attnT"], attnT.bitcast(F32))

            # final row-0 contributions (all chunk-0 matmuls as soon as head
            # lhA's normalize lands, chunk-1 after lhB's), then one paired
            # evacuation + store per psum tile
            for chunk in range(2):
                for i, nt in enumerate(range(4, 8)):
                    nc.tensor.matmul(
                        opsf[i // 2][:, (i % 2) * 512:(i % 2) * 512 + 256],
                        lhsT=attnT[0:32, chunk, nt * 128:(nt + 1) * 128],
                        rhs=outwT_sb[0:32, chunk, :],
                        start=False,
                        stop=(chunk == 1),
                        tile_position=(0, 0),
                    )
            for half_t, eng, q in ((0, nc.vector, nc.sync), (1, None, nc.scalar)):
                osb2 = outs_p.tile([128, 2, C], F32, tag="o", name="osb2")
                src = opsf[half_t].rearrange("p (t c) -> p t c", t=2)[:, :, 0:256]
                if eng is None:
                    nc.scalar.copy(osb2, src)
                else:
                    eng.tensor_copy(osb2, src)
                r0 = 512 + half_t * 256
                q.dma_start(
                    out_d[r0:r0 + 256, :].rearrange("(t p) c -> p t c", p=128),
                    osb2,
                )

    nc.compile()
    return nc


_NC = None
LAST_RESULTS = None


def _host_prep(conv_w, conv_b, qkv_w, out_w, out_b):
    conv_w = np.asarray(conv_w, np.float32).reshape(C, 3, 3)
    diag = np.zeros((2, 9, 128, 128), np.float32)
    idx = np.arange(128)
    for ct in range(2):
        for t, (ky, kx) in enumerate(TAPS):
            d = conv_w[128 * ct: 128 * (ct + 1), ky, kx].copy()
            if (ky, kx) == (1, 1):
                d += 1.0  # residual connection folded into the center tap
            diag[ct, t, idx, idx] = d
    return {
        "qkv_wT": np.ascontiguousarray(np.asarray(qkv_w, np.float32).T),
        "out_wT": np.ascontiguousarray(np.asarray(out_w, np.float32).T),
        "conv_diag": diag,
        "conv_b_r": np.asarray(conv_b, np.float32).reshape(1, C),
        "out_b_r": np.asarray(out_b, np.float32).reshape(1, C),
        "id128": np.eye(128, dtype=np.float32),
    }


def kernel(x, conv_w, conv_b, qkv_w, out_w, out_b):
    global _NC, LAST_RESULTS
    if _NC is None:
        _NC = build_nc()
    x = np.asarray(x, np.float32)
    shared = _host_prep(conv_w, conv_b, qkv_w, out_w, out_b)
    in_maps = [{**shared, "x": np.ascontiguousarray(x[b])} for b in range(B)]
    trace = bool(int(os.environ.get("KERNEL_TRACE", "0")))
    try:
        res = run_bass_kernel_spmd(_NC, in_maps, core_ids=list(range(B)), trace=trace)
    except Exception:
        if not trace:
            raise
        res = run_bass_kernel_spmd(_NC, in_maps, core_ids=list(range(B)), trace=False)
    LAST_RESULTS = res
    return np.stack([res.results[b]["out"] for b in range(B)], axis=0)
